# revision 1
# baseline (speedup 1.0000x reference)
"""Trainium2 Bass kernel for nn_LocalitySelfAttention.

The module's attention scores get +1e9 added on the diagonal before the
softmax (torch's ``attn - diag(-1e9)``).  QK^T scores for randn inputs are
O(1), so every softmax row is an exact fp32 one-hot at the diagonal and
``attn @ v == v`` bit-exactly.  The whole module therefore reduces to

    out = x @ Wv.T @ w_proj.T + b_proj,      Wv = w_qkv[512:768]

which is a memory-bound GEMM.  The kernel shards the 8192 (B*N) rows across
the 8 NeuronCores (1024 rows each).  Each core:

  1. folds W2T[k,p] = sum_vd Wv[vd,k] * w_proj[p,vd] on the TensorEngine
     (4 matmuls contracting vd),
  2. broadcasts b_proj across partitions with a stride-0 DMA,
  3. computes out[n,p] = sum_k xT[k,n] * W2T[k,p] + b[p] as 8 PSUM tiles
     (2 matmuls each, K=128), bias added during the PSUM->SBUF copy.

The host only moves bytes: it flattens/transposes x (the TensorEngine
contracts along the partition axis, so x must arrive k-major) and
concatenates the per-core row blocks of the output.

DMA order matters: the small weight tensors are issued first so the fold
can start while the 1 MB x slice streams in (in chunks, so the main
matmuls pipeline behind the DMA).
"""

import os
import sys

import numpy as np

if "/opt/trn_rl_repo" not in sys.path:
    sys.path.insert(0, "/opt/trn_rl_repo")

B, N, C = 2, 4096, 256
ROWS = B * N              # 8192
NCORES = 8
RPC = ROWS // NCORES      # 1024 rows per core
NT = RPC // 128           # 8 row-tiles of 128 per core
XCHUNKS = 2               # xt DMA split (pipelines DMA with matmuls)

# matmul operand dtype: float32r streams 1 row/cycle (vs 4 for float32)
USE_F32R = os.environ.get("K_F32R", "0") == "1"
# PE warmup matmuls issued while input DMAs are in flight
NWARM = int(os.environ.get("K_NWARM", "12"))

_cache = {}


def _build():
    """Build + compile the per-core Bass program (same program, SPMD)."""
    import concourse.bacc as bacc
    import concourse.bass as bass
    import concourse.mybir as mybir
    import concourse.tile as tile

    f32 = mybir.dt.float32
    mm_dt = mybir.dt.float32r if USE_F32R else f32

    def mm(ap):
        return ap.bitcast(mm_dt) if USE_F32R else ap

    nc = bacc.Bacc(
        "TRN2",
        target_bir_lowering=False,
        debug=False,
        num_devices=NCORES,
    )

    xt_d = nc.dram_tensor("xt", [C, RPC], f32, kind="ExternalInput")
    wv_d = nc.dram_tensor("wv", [C, C], f32, kind="ExternalInput")
    wpt_d = nc.dram_tensor("wpt", [C, C], f32, kind="ExternalInput")
    b_d = nc.dram_tensor("b", [C], f32, kind="ExternalInput")
    out_d = nc.dram_tensor("out", [RPC, C], f32, kind="ExternalOutput")

    xt = xt_d.ap()
    wv = wv_d.ap()
    wpt = wpt_d.ap()
    b = b_d.ap()
    out = out_d.ap()

    with tile.TileContext(nc) as tc:
        with (
            tc.tile_pool(name="const", bufs=1) as cp,
            tc.tile_pool(name="io", bufs=4) as io,
            tc.tile_pool(name="psw", bufs=2, space="PSUM") as psw,
            tc.tile_pool(name="pso", bufs=4, space="PSUM") as pso,
        ):
            # ---- small loads first: weights + bias ----
            # Wv natural [vd, k] -> [p(vd), vdc, k]
            wv_sb = cp.tile([128, 2, C], f32)
            nc.sync.dma_start(out=wv_sb, in_=wv.rearrange("(vdc p) k -> p vdc k", p=128))

            # w_proj^T [vd, p] -> [p(vd), vdc, pcol]
            wpt_sb = cp.tile([128, 2, C], f32)
            nc.sync.dma_start(out=wpt_sb, in_=wpt.rearrange("(vdc p) q -> p vdc q", p=128))

            # bias broadcast across all 128 partitions (stride-0 partition DMA)
            bias_bc = cp.tile([128, C], f32)
            b_bcast = bass.AP(
                tensor=b.tensor,
                offset=b.offset,
                ap=[[0, 128]] + [list(d) for d in b.ap],
            )
            nc.gpsimd.dma_start(out=bias_bc, in_=b_bcast)

            # ---- x^T slice, k-major: [k=256, n=1024] -> [p, kc, n], chunked ----
            xt_sb = cp.tile([128, 2, RPC], f32)
            xt_v = xt.rearrange("(kc p) n -> p kc n", p=128)
            csz = RPC // XCHUNKS
            for ch in range(XCHUNKS):
                nc.sync.dma_start(
                    out=xt_sb[:, :, ch * csz:(ch + 1) * csz],
                    in_=xt_v[:, :, ch * csz:(ch + 1) * csz],
                )

            # ---- PE warmup: dummy matmuls during the input-DMA wait so the
            # HAM clock gate reaches 2.4 GHz before the real work ----
            warm_sb = cp.tile([128, 128], f32)
            nc.vector.memset(warm_sb, 0.0)
            warm_ps = psw.tile([128, 128], f32, tag="warm")
            for _ in range(NWARM):
                nc.tensor.matmul(warm_ps, warm_sb, warm_sb, start=True, stop=True)

            # ---- fold W2T[k, p] = sum_vd Wv[vd, k] * wpt[vd, p] ----
            w2t_sb = cp.tile([128, 2, C], f32)  # [p(k), kc, pcol]
            for kc in range(2):
                ps = psw.tile([128, C], f32)
                for vdc in range(2):
                    nc.tensor.matmul(
                        ps,
                        wv_sb[:, vdc, kc * 128:(kc + 1) * 128],
                        wpt_sb[:, vdc, :],
                        start=(vdc == 0),
                        stop=(vdc == 1),
                    )
                nc.vector.tensor_copy(w2t_sb[:, kc, :], ps)

            # ---- main GEMM: out[n, p] = sum_k xT[k, n] * W2T[k, p] + b[p] ----
            out_v = out.rearrange("(t p) m -> p t m", p=128)
            for t in range(NT):
                ps = pso.tile([128, C], f32)
                nc.tensor.matmul(
                    ps, mm(xt_sb[:, 0, t * 128:(t + 1) * 128]), mm(w2t_sb[:, 0, :]),
                    start=True, stop=False,
                )
                nc.tensor.matmul(
                    ps, mm(xt_sb[:, 1, t * 128:(t + 1) * 128]), mm(w2t_sb[:, 1, :]),
                    start=False, stop=True,
                )
                ot = io.tile([128, C], f32)
                nc.vector.tensor_add(ot, ps, bias_bc)
                nc.sync.dma_start(out=out_v[:, t, :], in_=ot)

    nc.compile()
    return nc


def run_sharded(inputs, trace=False, trace_cores=None):
    """Shard inputs, run on the 8 NeuronCores, gather.  Returns
    (full_output, BassKernelResults)."""
    from concourse.bass_utils import run_bass_kernel_spmd

    x = np.ascontiguousarray(np.asarray(inputs["x"], dtype=np.float32))
    w_qkv = np.ascontiguousarray(np.asarray(inputs["w_qkv"], dtype=np.float32))
    w_proj = np.ascontiguousarray(np.asarray(inputs["w_proj"], dtype=np.float32))
    b_proj = np.ascontiguousarray(np.asarray(inputs["b_proj"], dtype=np.float32))

    if "nc" not in _cache:
        _cache["nc"] = _build()
    nc = _cache["nc"]

    # host-side layout marshaling only (no FLOPs)
    xT = np.ascontiguousarray(x.reshape(ROWS, C).T)          # [256, 8192]
    wv = np.ascontiguousarray(w_qkv[2 * C:3 * C])            # [256, 256]
    wpt = np.ascontiguousarray(w_proj.T)                     # [256, 256]

    in_maps = [
        {
            "xt": np.ascontiguousarray(xT[:, c * RPC:(c + 1) * RPC]),
            "wv": wv,
            "wpt": wpt,
            "b": b_proj,
        }
        for c in range(NCORES)
    ]

    res = run_bass_kernel_spmd(
        nc,
        in_maps,
        core_ids=list(range(NCORES)),
        trace=trace,
        trace_cores=trace_cores,
    )
    out = np.concatenate(
        [res.results[c]["out"] for c in range(NCORES)], axis=0
    )  # [8192, 256]
    return out.reshape(B, N, C), res


def kernel(x, w_qkv, w_proj, b_proj, temperature):
    out, _ = run_sharded(
        {"x": x, "w_qkv": w_qkv, "w_proj": w_proj, "b_proj": b_proj}
    )
    return out



# revision 2
# speedup vs baseline: 1.1861x; 1.1861x over previous
"""Trainium2 Bass kernel for nn_LocalitySelfAttention.

The module's attention scores get +1e9 added on the diagonal before the
softmax (torch's ``attn - diag(-1e9)``).  QK^T scores for randn inputs are
O(1), so every softmax row is an exact fp32 one-hot at the diagonal and
``attn @ v == v`` bit-exactly.  The whole module therefore reduces to

    out = x @ Wv.T @ w_proj.T + b_proj,      Wv = w_qkv[512:768]

which is a memory-bound GEMM.  The kernel shards the 8192 (B*N) rows across
the 8 NeuronCores (1024 rows each).

v2 (this file): all operands stream in bf16 (rel err ~3.4e-3, gate 2e-2),
which halves HBM bytes and quadruples PE streaming rate vs f32.  DMAs are
split across BOTH HWDGE rings (qSP via nc.sync, qAct via nc.scalar) since a
single ring sustains only ~167 GB/s.  DRAM tensors are laid out as exact
SBUF images (host does all reshapes) so every DMA descriptor is a maximal
contiguous run.  Each core:

  1. loads Wv / w_proj^T (bf16, one per ring), then x^T in chunks
     (alternating rings), bias f32 via a stride-0 gpsimd broadcast,
  2. folds W2T[k,p] = sum_vd Wv[vd,k] * w_proj^T[vd,p] on the PE
     (4 bf16 matmuls into one PSUM bank), copies to SBUF as bf16,
  3. computes out[n,p] = sum_k xT[k,n] * W2T[k,p] + b[p]: per pair of
     128-row tiles one PSUM bank [128,512] filled by 4 matmuls, one DVE
     add (bias + f32->bf16 cast), one out DMA per pair (rings alternate).

The host only moves bytes / casts dtypes: slice Wv, transpose, bf16-cast,
and re-assemble the output tiles.
"""

import os
import sys

import numpy as np

if "/opt/trn_rl_repo" not in sys.path:
    sys.path.insert(0, "/opt/trn_rl_repo")

import ml_dtypes

BF16 = ml_dtypes.bfloat16

B, N, C = 2, 4096, 256
ROWS = B * N              # 8192
NCORES = 8
RPC = ROWS // NCORES      # 1024 rows per core
NT = RPC // 128           # 8 row-tiles of 128 per core
XCH = 4                   # x DMA chunks (2 per ring)
NPAIR = NT // 2           # 4 output pairs

# PE warmup matmuls issued while input DMAs are in flight
NWARM = int(os.environ.get("K_NWARM", "4"))

_cache = {}


def _build():
    """Build + compile the per-core Bass program (same program, SPMD)."""
    import concourse.bacc as bacc
    import concourse.bass as bass
    import concourse.mybir as mybir
    import concourse.tile as tile

    f32 = mybir.dt.float32
    bf16 = mybir.dt.bfloat16

    nc = bacc.Bacc(
        "TRN2",
        target_bir_lowering=False,
        debug=False,
        num_devices=NCORES,
    )

    # DRAM tensors are SBUF images: [128 partitions, free] with the free
    # dim laid out exactly as the SBUF tile, so DMA descriptors are
    # maximal contiguous runs per partition.
    xt_d = nc.dram_tensor("xt", [128, 2 * RPC], bf16, kind="ExternalInput")
    wv_d = nc.dram_tensor("wv", [128, 2 * C], bf16, kind="ExternalInput")
    wpt_d = nc.dram_tensor("wpt", [128, 2 * C], bf16, kind="ExternalInput")
    b_d = nc.dram_tensor("b", [C], f32, kind="ExternalInput")
    out_d = nc.dram_tensor("out", [128, NT * C], bf16, kind="ExternalOutput")

    xt = xt_d.ap()
    wv = wv_d.ap()
    wpt = wpt_d.ap()
    b = b_d.ap()
    out = out_d.ap()

    with tile.TileContext(nc) as tc:
        with (
            tc.tile_pool(name="const", bufs=1) as cp,
            tc.tile_pool(name="io", bufs=4) as io,
            tc.tile_pool(name="psw", bufs=2, space="PSUM") as psw,
            tc.tile_pool(name="pso", bufs=4, space="PSUM") as pso,
        ):
            # ---- weights first: one per HWDGE ring ----
            wv_sb = cp.tile([128, 2, C], bf16)   # [p(vd), vdc, k]
            nc.sync.dma_start(out=wv_sb, in_=wv.rearrange("p (vdc k) -> p vdc k", vdc=2))
            wpt_sb = cp.tile([128, 2, C], bf16)  # [p(vd), vdc, pcol]
            nc.scalar.dma_start(out=wpt_sb, in_=wpt.rearrange("p (vdc k) -> p vdc k", vdc=2))

            # bias broadcast across all 128 partitions (stride-0 partition
            # DMA, SWDGE) -- off the HWDGE rings and off the critical path
            bias_bc = cp.tile([128, C], f32)
            b_bcast = bass.AP(
                tensor=b.tensor,
                offset=b.offset,
                ap=[[0, 128]] + [list(d) for d in b.ap],
            )
            nc.gpsimd.dma_start(out=bias_bc, in_=b_bcast)

            # ---- x^T slice, chunked across both rings ----
            # DRAM image: col = ch*(2*CS) + kc*CS + nn   (CS = RPC//XCH)
            # SBUF tile:  [p, kc, n] with n = ch*CS + nn
            xt_sb = cp.tile([128, 2, RPC], bf16)
            CS = RPC // XCH
            xt_v = xt.rearrange("p (ch kc nn) -> p ch kc nn", ch=XCH, kc=2)
            for ch in range(XCH):
                eng = nc.sync if ch % 2 == 0 else nc.scalar
                eng.dma_start(
                    out=xt_sb[:, :, ch * CS:(ch + 1) * CS],
                    in_=xt_v[:, ch],
                )

            # ---- PE warmup: keep the PE busy from t~0 so the HAM clock
            # gate's busy window starts as early as possible ----
            warm_sb = cp.tile([128, 128], bf16)
            nc.vector.memset(warm_sb, 0.0)
            warm_ps = psw.tile([128, 128], f32, tag="warm")
            for _ in range(NWARM):
                nc.tensor.matmul(warm_ps, warm_sb, warm_sb, start=True, stop=True)

            # ---- fold W2T[k, p] = sum_vd Wv[vd, k] * wpt[vd, p] ----
            # both k-chunks go into one PSUM bank [128, 512]
            w2t_sb = cp.tile([128, 2, C], bf16)  # [p(k), kc, pcol]
            ps_f = psw.tile([128, 2, C], f32)
            for kc in range(2):
                for vdc in range(2):
                    nc.tensor.matmul(
                        ps_f[:, kc, :],
                        wv_sb[:, vdc, kc * 128:(kc + 1) * 128],
                        wpt_sb[:, vdc, :],
                        start=(vdc == 0),
                        stop=(vdc == 1),
                    )
            nc.vector.tensor_copy(w2t_sb, ps_f)

            # ---- main GEMM: out[n, p] = sum_k xT[k, n] * W2T[k, p] + b[p]
            # one PSUM bank per PAIR of row-tiles -> 1 DVE add + 1 DMA per pair
            out_v = out.rearrange("p (t m) -> p t m", t=NT)
            for pr in range(NPAIR):
                ps = pso.tile([128, 2, C], f32)
                for half in range(2):
                    t = 2 * pr + half
                    for kc in range(2):
                        nc.tensor.matmul(
                            ps[:, half, :],
                            xt_sb[:, kc, t * 128:(t + 1) * 128],
                            w2t_sb[:, kc, :],
                            start=(kc == 0),
                            stop=(kc == 1),
                        )
                ot = io.tile([128, 2, C], bf16)
                for half in range(2):
                    nc.vector.tensor_add(ot[:, half, :], ps[:, half, :], bias_bc)
                eng = nc.sync if pr % 2 == 0 else nc.scalar
                eng.dma_start(out=out_v[:, 2 * pr:2 * pr + 2, :], in_=ot)

    nc.compile()
    return nc


def run_sharded(inputs, trace=False, trace_cores=None):
    """Shard inputs, run on the 8 NeuronCores, gather.  Returns
    (full_output, BassKernelResults)."""
    from concourse.bass_utils import run_bass_kernel_spmd

    x = np.asarray(inputs["x"], dtype=np.float32)
    w_qkv = np.asarray(inputs["w_qkv"], dtype=np.float32)
    w_proj = np.asarray(inputs["w_proj"], dtype=np.float32)
    b_proj = np.ascontiguousarray(np.asarray(inputs["b_proj"], dtype=np.float32))

    if "nc" not in _cache:
        _cache["nc"] = _build()
    nc = _cache["nc"]

    # host-side layout marshaling + bf16 cast only (no FLOPs)
    xT = x.reshape(ROWS, C).T.astype(BF16)                   # [256, 8192]
    # weight SBUF images: [vd, c] -> [p, vdc, c] -> [128, 512]
    wv = w_qkv[2 * C:3 * C].astype(BF16)
    wv_img = np.ascontiguousarray(
        wv.reshape(2, 128, C).transpose(1, 0, 2).reshape(128, 2 * C))
    wpt = w_proj.T.astype(BF16)
    wpt_img = np.ascontiguousarray(
        wpt.reshape(2, 128, C).transpose(1, 0, 2).reshape(128, 2 * C))

    CS = RPC // XCH
    in_maps = []
    for c in range(NCORES):
        xc = xT[:, c * RPC:(c + 1) * RPC]                    # [256, 1024]
        # [kc*128+p, ch*CS+nn] -> img[p, ch, kc, nn]
        xc_img = np.ascontiguousarray(
            xc.reshape(2, 128, XCH, CS).transpose(1, 2, 0, 3).reshape(128, 2 * RPC))
        in_maps.append({"xt": xc_img, "wv": wv_img, "wpt": wpt_img, "b": b_proj})

    res = run_bass_kernel_spmd(
        nc,
        in_maps,
        core_ids=list(range(NCORES)),
        trace=trace,
        trace_cores=trace_cores,
    )
    # out image [128, t, m] -> rows t*128+p of the core's [1024, 256] block
    blocks = [
        res.results[c]["out"].reshape(128, NT, C).transpose(1, 0, 2).reshape(RPC, C)
        for c in range(NCORES)
    ]
    out = np.concatenate(blocks, axis=0).astype(np.float32)  # [8192, 256]
    return out.reshape(B, N, C), res


def kernel(x, w_qkv, w_proj, b_proj, temperature):
    out, _ = run_sharded(
        {"x": x, "w_qkv": w_qkv, "w_proj": w_proj, "b_proj": b_proj}
    )
    return out


# revision 3
# speedup vs baseline: 1.2208x; 1.0293x over previous
"""Trainium2 Bass kernel for nn_LocalitySelfAttention.

The module's attention scores get +1e9 added on the diagonal before the
softmax (torch's ``attn - diag(-1e9)``).  QK^T scores for randn inputs are
O(1), so every softmax row is an exact fp32 one-hot at the diagonal and
``attn @ v == v`` bit-exactly.  The whole module therefore reduces to

    out = x @ Wv.T @ w_proj.T + b_proj,      Wv = w_qkv[512:768]

a memory-bound GEMM, sharded 1024 rows per NeuronCore.

v3 notes (from trace analysis of v1/v2):
  * All operands bf16 (rel err ~3.4e-3 vs the 2e-2 gate): halves HBM bytes,
    4x PE streaming rate vs f32.
  * One HWDGE ring sustains only ~170-250 GB/s and each DMA_DIRECT2D
    trigger occupies its issuing engine ~0.7 us, so: both HWDGE rings
    (nc.sync -> qSP, nc.scalar -> qAct) are used, with FEW, LARGE DMAs.
  * DMA descriptor runs below ~1 KB tank throughput (512 B runs measured
    ~92 GB/s vs ~250 at 1 KB+).  All DRAM tensors are exact SBUF images
    (host does every reshape/cast), so each DMA is contiguous per
    partition: ring A blob = wv | bias | x0 | x2, ring B = wpt | x1 | x3.
  * Weights ride at the HEAD of each ring so the fold can start ~1 us
    before x finishes streaming; the fold's PSUM->SBUF copy is split per
    k-chunk so the first main matmuls start one copy earlier.
  * The PE is clock-gated at 1.2 GHz until ~3.4 us of sustained activity:
    warmup matmuls start at t~0 so the main GEMM mostly runs at 2.4 GHz.
  * Output pairs (2 row-tiles = [128,512] bf16) each take 4 matmuls into
    one PSUM bank, 2 DVE bias-adds (f32->bf16 cast), 1 DMA, rings
    alternating.
"""

import os
import sys

import numpy as np

if "/opt/trn_rl_repo" not in sys.path:
    sys.path.insert(0, "/opt/trn_rl_repo")

import ml_dtypes

BF16 = ml_dtypes.bfloat16

B, N, C = 2, 4096, 256
ROWS = B * N              # 8192
NCORES = 8
RPC = ROWS // NCORES      # 1024 rows per core
NT = RPC // 128           # 8 row-tiles of 128 per core
NPAIR = NT // 2           # 4 output pairs
XCH = 4                   # x chunks (2 per ring)
CS = RPC // XCH           # 256 n-columns per chunk

# column layout (bf16 elements) of the two input blobs
#   inA: wv[0:512] | bias[512:768] | x0[768:1280] | x2[1280:1792]
#   inB: wpt[0:512] | x1[512:1024] | x3[1024:1536]
WA = 2 * C + C + 2 * 2 * CS   # 1792
WB = 2 * C + 2 * 2 * CS       # 1536
A_BIAS = 2 * C                # 512
A_X = {0: A_BIAS + C, 2: A_BIAS + C + 2 * CS}   # 768, 1280
B_X = {1: 2 * C, 3: 2 * C + 2 * CS}             # 512, 1024

NWARM = int(os.environ.get("K_NWARM", "8"))

_cache = {}


def _build():
    """Build + compile the per-core Bass program (same program, SPMD)."""
    import concourse.bacc as bacc
    import concourse.mybir as mybir
    import concourse.tile as tile

    f32 = mybir.dt.float32
    bf16 = mybir.dt.bfloat16

    nc = bacc.Bacc(
        "TRN2",
        target_bir_lowering=False,
        debug=False,
        num_devices=NCORES,
    )

    inA_d = nc.dram_tensor("inA", [128, WA], bf16, kind="ExternalInput")
    inB_d = nc.dram_tensor("inB", [128, WB], bf16, kind="ExternalInput")
    out_d = nc.dram_tensor("out", [128, NT * C], bf16, kind="ExternalOutput")

    inA = inA_d.ap()
    inB = inB_d.ap()
    out = out_d.ap()

    with tile.TileContext(nc) as tc:
        with (
            tc.tile_pool(name="const", bufs=1) as cp,
            tc.tile_pool(name="io", bufs=4) as io,
            tc.tile_pool(name="psw", bufs=2, space="PSUM") as psw,
            tc.tile_pool(name="pso", bufs=4, space="PSUM") as pso,
        ):
            sbA = cp.tile([128, WA], bf16)
            sbB = cp.tile([128, WB], bf16)

            # ring A: weights+bias first, then x chunks 0, 2
            nc.sync.dma_start(out=sbA[:, :A_X[0]], in_=inA[:, :A_X[0]])
            # ring B: wpt, then x chunks 1, 3
            nc.scalar.dma_start(out=sbB[:, :B_X[1]], in_=inB[:, :B_X[1]])
            nc.sync.dma_start(out=sbA[:, A_X[0]:A_X[2]], in_=inA[:, A_X[0]:A_X[2]])
            nc.scalar.dma_start(out=sbB[:, B_X[1]:B_X[3]], in_=inB[:, B_X[1]:B_X[3]])
            nc.sync.dma_start(out=sbA[:, A_X[2]:], in_=inA[:, A_X[2]:])
            nc.scalar.dma_start(out=sbB[:, B_X[3]:], in_=inB[:, B_X[3]:])

            # PE warmup: busy from t~0 so the HAM busy-window (3.4 us to
            # ungate 1.2 -> 2.4 GHz) elapses before the main GEMM
            warm_sb = cp.tile([128, 128], bf16)
            nc.vector.memset(warm_sb, 0.0)
            warm_ps = psw.tile([128, 128], f32, tag="warm")
            for _ in range(NWARM):
                nc.tensor.matmul(warm_ps, warm_sb, warm_sb, start=True, stop=True)

            # fold W2T[k,p] = sum_vd Wv[vd,k] * wpt[vd,p]; copy per k-chunk
            w2t = cp.tile([128, 2 * C], bf16)     # [p(k), kc*256 + pcol]
            ps_f = psw.tile([128, 2 * C], f32)
            for kc in range(2):
                for vdc in range(2):
                    nc.tensor.matmul(
                        ps_f[:, kc * C:(kc + 1) * C],
                        sbA[:, vdc * C + kc * 128: vdc * C + kc * 128 + 128],
                        sbB[:, vdc * C:(vdc + 1) * C],
                        start=(vdc == 0),
                        stop=(vdc == 1),
                    )
                nc.vector.tensor_copy(
                    w2t[:, kc * C:(kc + 1) * C], ps_f[:, kc * C:(kc + 1) * C])

            # main GEMM, one PSUM bank per pair of row-tiles
            bias = sbA[:, A_BIAS:A_BIAS + C]
            out_v = out.rearrange("p (pr m) -> p pr m", pr=NPAIR)
            for pr in range(NPAIR):
                ps = pso.tile([128, 2 * C], f32)
                for half in range(2):
                    t = 2 * pr + half
                    ch = t // 2
                    base = A_X[ch] if ch % 2 == 0 else B_X[ch]
                    sb = sbA if ch % 2 == 0 else sbB
                    for kc in range(2):
                        nc.tensor.matmul(
                            ps[:, half * C:(half + 1) * C],
                            sb[:, base + kc * CS + (t % 2) * 128:
                                  base + kc * CS + (t % 2) * 128 + 128],
                            w2t[:, kc * C:(kc + 1) * C],
                            start=(kc == 0),
                            stop=(kc == 1),
                        )
                ot = io.tile([128, 2 * C], bf16)
                for half in range(2):
                    nc.vector.tensor_add(
                        ot[:, half * C:(half + 1) * C],
                        ps[:, half * C:(half + 1) * C], bias)
                eng = nc.sync if pr % 2 == 0 else nc.scalar
                eng.dma_start(out=out_v[:, pr, :], in_=ot)

    nc.compile()
    return nc


def run_sharded(inputs, trace=False, trace_cores=None):
    """Shard inputs, run on the 8 NeuronCores, gather.  Returns
    (full_output, BassKernelResults)."""
    from concourse.bass_utils import run_bass_kernel_spmd

    x = np.asarray(inputs["x"], dtype=np.float32)
    w_qkv = np.asarray(inputs["w_qkv"], dtype=np.float32)
    w_proj = np.asarray(inputs["w_proj"], dtype=np.float32)
    b_proj = np.asarray(inputs["b_proj"], dtype=np.float32)

    if "nc" not in _cache:
        _cache["nc"] = _build()
    nc = _cache["nc"]

    # host-side layout marshaling + bf16 cast only (no FLOPs)
    xT = x.reshape(ROWS, C).T.astype(BF16)                   # [256, 8192]

    def img(w):  # [vd, c] (vdc-major) -> SBUF image [128, 512]
        return w.reshape(2, 128, C).transpose(1, 0, 2).reshape(128, 2 * C)

    wv_img = img(w_qkv[2 * C:3 * C].astype(BF16))
    wpt_img = img(np.ascontiguousarray(w_proj.T).astype(BF16))
    bias16 = np.broadcast_to(b_proj.astype(BF16), (128, C))

    in_maps = []
    for c in range(NCORES):
        xc = xT[:, c * RPC:(c + 1) * RPC]                    # [256, 1024]
        chunks = [
            xc[:, ch * CS:(ch + 1) * CS]
            .reshape(2, 128, CS).transpose(1, 0, 2).reshape(128, 2 * CS)
            for ch in range(XCH)
        ]
        inA = np.ascontiguousarray(
            np.concatenate([wv_img, bias16, chunks[0], chunks[2]], axis=1))
        inB = np.ascontiguousarray(
            np.concatenate([wpt_img, chunks[1], chunks[3]], axis=1))
        in_maps.append({"inA": inA, "inB": inB})

    res = run_bass_kernel_spmd(
        nc,
        in_maps,
        core_ids=list(range(NCORES)),
        trace=trace,
        trace_cores=trace_cores,
    )
    # out image [128, t, m] -> rows t*128+p of the core's [1024, 256] block
    blocks = [
        res.results[c]["out"].reshape(128, NT, C).transpose(1, 0, 2).reshape(RPC, C)
        for c in range(NCORES)
    ]
    out = np.concatenate(blocks, axis=0).astype(np.float32)  # [8192, 256]
    return out.reshape(B, N, C), res


def kernel(x, w_qkv, w_proj, b_proj, temperature):
    out, _ = run_sharded(
        {"x": x, "w_qkv": w_qkv, "w_proj": w_proj, "b_proj": b_proj}
    )
    return out


# revision 9
# speedup vs baseline: 1.2990x; 1.0640x over previous
"""Trainium2 Bass kernel for nn_LocalitySelfAttention.

The module's attention scores get +1e9 added on the diagonal before the
softmax (torch's ``attn - diag(-1e9)``).  QK^T scores for randn inputs are
O(1), so every softmax row is an exact fp32 one-hot at the diagonal and
``attn @ v == v`` bit-exactly.  The whole module therefore reduces to

    out = x @ Wv.T @ w_proj.T + b_proj,      Wv = w_qkv[512:768]

a memory-bound GEMM, sharded 1024 rows per NeuronCore.

v3 notes (from trace analysis of v1/v2):
  * All operands bf16 (rel err ~3.4e-3 vs the 2e-2 gate): halves HBM bytes,
    4x PE streaming rate vs f32.
  * One HWDGE ring sustains only ~170-250 GB/s and each DMA_DIRECT2D
    trigger occupies its issuing engine ~0.7 us, so: both HWDGE rings
    (nc.sync -> qSP, nc.scalar -> qAct) are used, with FEW, LARGE DMAs.
  * DMA descriptor runs below ~1 KB tank throughput (512 B runs measured
    ~92 GB/s vs ~250 at 1 KB+).  All DRAM tensors are exact SBUF images
    (host does every reshape/cast), so each DMA is contiguous per
    partition: ring A blob = wv | bias | x0 | x2, ring B = wpt | x1 | x3.
  * Weights ride at the HEAD of each ring so the fold can start ~1 us
    before x finishes streaming; the fold's PSUM->SBUF copy is split per
    k-chunk so the first main matmuls start one copy earlier.
  * The PE is clock-gated at 1.2 GHz until ~3.4 us of sustained activity:
    warmup matmuls start at t~0 so the main GEMM mostly runs at 2.4 GHz.
  * Output pairs (2 row-tiles = [128,512] bf16) each take 4 matmuls into
    one PSUM bank, 2 DVE bias-adds (f32->bf16 cast), 1 DMA, rings
    alternating.
"""

import os
import sys

import numpy as np

if "/opt/trn_rl_repo" not in sys.path:
    sys.path.insert(0, "/opt/trn_rl_repo")

import ml_dtypes

BF16 = ml_dtypes.bfloat16

B, N, C = 2, 4096, 256
ROWS = B * N              # 8192
NCORES = 8
RPC = ROWS // NCORES      # 1024 rows per core
NT = RPC // 128           # 8 row-tiles of 128 per core
NPAIR = NT // 2           # 4 output pairs
XCH = 4                   # x chunks (2 per ring)
CS = RPC // XCH           # 256 n-columns per chunk

# column layout (bf16 elements) of the three input blobs
#   inA (qSP ring):   wv[0:512] | bias[512:768] | x0[768:1280]
#   inB (qAct ring):  wpt[0:512] | x1[512:1024] | x3[1024:1536]
#   inC (gpsimd/SWDGE): x2[0:512]
WA = 2 * C + C + 2 * CS       # 1280
WB = 2 * C + 2 * 2 * CS       # 1536
WC = 2 * CS                   # 512
A_BIAS = 2 * C                # 512
A_X = {0: A_BIAS + C}         # 768
B_X = {1: 2 * C, 3: 2 * C + 2 * CS}             # 512, 1024
C_X = {2: 0}

NWARM = int(os.environ.get("K_NWARM", "26"))

_cache = {}


def _build():
    """Build + compile the per-core Bass program (same program, SPMD)."""
    import concourse.bacc as bacc
    import concourse.mybir as mybir
    import concourse.tile as tile

    f32 = mybir.dt.float32
    bf16 = mybir.dt.bfloat16

    nc = bacc.Bacc(
        "TRN2",
        target_bir_lowering=False,
        debug=False,
        num_devices=NCORES,
    )

    inA_d = nc.dram_tensor("inA", [128, WA], bf16, kind="ExternalInput")
    inB_d = nc.dram_tensor("inB", [128, WB], bf16, kind="ExternalInput")
    inC_d = nc.dram_tensor("inC", [128, WC], bf16, kind="ExternalInput")
    out_d = nc.dram_tensor("out", [128, NT * C], bf16, kind="ExternalOutput")

    inA = inA_d.ap()
    inB = inB_d.ap()
    inC = inC_d.ap()
    out = out_d.ap()

    with tile.TileContext(nc) as tc:
        with (
            tc.tile_pool(name="const", bufs=1) as cp,
            tc.tile_pool(name="io", bufs=4) as io,
            tc.tile_pool(name="psw", bufs=2, space="PSUM") as psw,
            tc.tile_pool(name="pso", bufs=4, space="PSUM") as pso,
        ):
            sbA = cp.tile([128, WA], bf16)
            sbB = cp.tile([128, WB], bf16)
            sbC = cp.tile([128, WC], bf16)

            # ring A: weights+bias first, then x chunk 0
            nc.sync.dma_start(out=sbA[:, :A_X[0]], in_=inA[:, :A_X[0]])
            # ring B: wpt, then x chunks 1, 3
            nc.scalar.dma_start(out=sbB[:, :B_X[1]], in_=inB[:, :B_X[1]])
            # 3rd stream: x chunk 2 via SWDGE
            nc.gpsimd.dma_start(out=sbC, in_=inC)
            nc.sync.dma_start(out=sbA[:, A_X[0]:], in_=inA[:, A_X[0]:])
            nc.scalar.dma_start(out=sbB[:, B_X[1]:B_X[3]], in_=inB[:, B_X[1]:B_X[3]])
            nc.scalar.dma_start(out=sbB[:, B_X[3]:], in_=inB[:, B_X[3]:])

            # PE warmup: busy from t~0 so the HAM busy-window (3.4 us to
            # ungate 1.2 -> 2.4 GHz) elapses before the main GEMM
            warm_sb = cp.tile([128, 128], bf16)
            nc.vector.memset(warm_sb, 0.0)
            warm_ps = psw.tile([128, 128], f32, tag="warm")
            for _ in range(NWARM):
                nc.tensor.matmul(warm_ps, warm_sb, warm_sb, start=True, stop=True)

            # fold W2T[k,p] = sum_vd Wv[vd,k] * wpt[vd,p]; separate PSUM
            # tile per k-chunk (shared tile would WAR-serialize kc1 matmuls
            # behind the kc0 DVE copy)
            w2t = cp.tile([128, 2 * C], bf16)     # [p(k), kc*256 + pcol]
            for kc in range(2):
                ps_f = psw.tile([128, C], f32)
                for vdc in range(2):
                    nc.tensor.matmul(
                        ps_f,
                        sbA[:, vdc * C + kc * 128: vdc * C + kc * 128 + 128],
                        sbB[:, vdc * C:(vdc + 1) * C],
                        start=(vdc == 0),
                        stop=(vdc == 1),
                    )
                nc.vector.tensor_copy(w2t[:, kc * C:(kc + 1) * C], ps_f)

            # main GEMM, one PSUM bank per pair of row-tiles
            bias = sbA[:, A_BIAS:A_BIAS + C]
            out_v = out.rearrange("p (pr m) -> p pr m", pr=NPAIR)
            for pr in range(NPAIR):
                ps = pso.tile([128, 2 * C], f32)
                for half in range(2):
                    t = 2 * pr + half
                    ch = t // 2
                    if ch in A_X:
                        base, sb = A_X[ch], sbA
                    elif ch in B_X:
                        base, sb = B_X[ch], sbB
                    else:
                        base, sb = C_X[ch], sbC
                    for kc in range(2):
                        nc.tensor.matmul(
                            ps[:, half * C:(half + 1) * C],
                            sb[:, base + kc * CS + (t % 2) * 128:
                                  base + kc * CS + (t % 2) * 128 + 128],
                            w2t[:, kc * C:(kc + 1) * C],
                            start=(kc == 0),
                            stop=(kc == 1),
                        )
                ot = io.tile([128, 2 * C], bf16)
                for half in range(2):
                    nc.vector.tensor_add(
                        ot[:, half * C:(half + 1) * C],
                        ps[:, half * C:(half + 1) * C], bias)
                eng = nc.sync if pr % 2 == 0 else nc.scalar
                eng.dma_start(out=out_v[:, pr, :], in_=ot)

    nc.compile()
    return nc


def run_sharded(inputs, trace=False, trace_cores=None):
    """Shard inputs, run on the 8 NeuronCores, gather.  Returns
    (full_output, BassKernelResults)."""
    from concourse.bass_utils import run_bass_kernel_spmd

    x = np.asarray(inputs["x"], dtype=np.float32)
    w_qkv = np.asarray(inputs["w_qkv"], dtype=np.float32)
    w_proj = np.asarray(inputs["w_proj"], dtype=np.float32)
    b_proj = np.asarray(inputs["b_proj"], dtype=np.float32)

    if "nc" not in _cache:
        _cache["nc"] = _build()
    nc = _cache["nc"]

    # host-side layout marshaling + bf16 cast only (no FLOPs)
    xT = x.reshape(ROWS, C).T.astype(BF16)                   # [256, 8192]

    def img(w):  # [vd, c] (vdc-major) -> SBUF image [128, 512]
        return w.reshape(2, 128, C).transpose(1, 0, 2).reshape(128, 2 * C)

    wv_img = img(w_qkv[2 * C:3 * C].astype(BF16))
    wpt_img = img(np.ascontiguousarray(w_proj.T).astype(BF16))
    bias16 = np.broadcast_to(b_proj.astype(BF16), (128, C))

    in_maps = []
    for c in range(NCORES):
        xc = xT[:, c * RPC:(c + 1) * RPC]                    # [256, 1024]
        chunks = [
            xc[:, ch * CS:(ch + 1) * CS]
            .reshape(2, 128, CS).transpose(1, 0, 2).reshape(128, 2 * CS)
            for ch in range(XCH)
        ]
        inA = np.ascontiguousarray(
            np.concatenate([wv_img, bias16, chunks[0]], axis=1))
        inB = np.ascontiguousarray(
            np.concatenate([wpt_img, chunks[1], chunks[3]], axis=1))
        inC = np.ascontiguousarray(chunks[2])
        in_maps.append({"inA": inA, "inB": inB, "inC": inC})

    res = run_bass_kernel_spmd(
        nc,
        in_maps,
        core_ids=list(range(NCORES)),
        trace=trace,
        trace_cores=trace_cores,
    )
    # out image [128, t, m] -> rows t*128+p of the core's [1024, 256] block
    blocks = [
        res.results[c]["out"].reshape(128, NT, C).transpose(1, 0, 2).reshape(RPC, C)
        for c in range(NCORES)
    ]
    out = np.concatenate(blocks, axis=0).astype(np.float32)  # [8192, 256]
    return out.reshape(B, N, C), res


def kernel(x, w_qkv, w_proj, b_proj, temperature):
    out, _ = run_sharded(
        {"x": x, "w_qkv": w_qkv, "w_proj": w_proj, "b_proj": b_proj}
    )
    return out


# revision 11
# speedup vs baseline: 1.3194x; 1.0157x over previous
"""Trainium2 Bass kernel for nn_LocalitySelfAttention.

The module's attention scores get +1e9 added on the diagonal before the
softmax (torch's ``attn - diag(-1e9)``).  QK^T scores for randn inputs are
O(1), so every softmax row is an exact fp32 one-hot at the diagonal and
``attn @ v == v`` bit-exactly.  The whole module therefore reduces to

    out = x @ Wv.T @ w_proj.T + b_proj,      Wv = w_qkv[512:768]

a memory-bound GEMM, sharded 1024 rows per NeuronCore.

v5 notes (trace-driven):
  * all operands bf16 (rel err ~3.4e-3 vs 2e-2 gate)
  * HWDGE rings read HBM at only ~130-155 GB/s each and every DMA trigger
    costs ~0.7 us of its issuing engine, so x is spread over THREE DGE
    streams (qSP, qAct, SWDGE) as few large SBUF-image DMAs (>=1KB runs)
  * weights ride alone at each ring head (wpt first - its ring also
    carries x1) so the fold starts as early as possible
  * the PE is clock-gated to 1.2 GHz until ~3.4 us of sustained activity;
    warmup matmuls bridge from the entry barrier to the fold so the main
    GEMM runs at 2.4 GHz
  * per pair of row-tiles: one PSUM bank, kc0-matmuls for both halves
    issued before kc1 (hides the second fold cast), ONE fused DVE
    bias-add over [128,2x256] with a stride-0 broadcast bias view,
    one out DMA (rings alternate)
  * fold PSUM->SBUF casts run on the Scalar (ACT) engine, keeping the
    DVE free for the pair bias-adds (the previous tail bottleneck)
"""

import os
import sys

import numpy as np

if "/opt/trn_rl_repo" not in sys.path:
    sys.path.insert(0, "/opt/trn_rl_repo")

import ml_dtypes

BF16 = ml_dtypes.bfloat16

B, N, C = 2, 4096, 256
ROWS = B * N              # 8192
NCORES = 8
RPC = ROWS // NCORES      # 1024 rows per core
NT = RPC // 128           # 8 row-tiles of 128 per core
NPAIR = NT // 2           # 4 output pairs
CS = 256                  # n-columns per x chunk (4 chunks)

# input blob column layouts (bf16 elements)
#   inA (qSP):   wv[0:512]  | bias[512:768] | x0[768:1280]
#   inB (qAct):  wpt[0:512] | x1[512:1024]
#   inC (SWDGE): x2[0:512]  | x3[512:1024]
WA, WB, WC = 1280, 1024, 1024
A_BIAS = 512
X_LOC = {0: ("A", 768), 1: ("B", 512), 2: ("C", 0), 3: ("C", 512)}
PAIR_ORDER = [1, 0, 2, 3]          # by expected x-chunk arrival
FUSED_BIAS = os.environ.get("K_FUSED_BIAS", "1") == "1"

NWARM = int(os.environ.get("K_NWARM", "24"))

_cache = {}


def _build():
    """Build + compile the per-core Bass program (same program, SPMD)."""
    import concourse.bacc as bacc
    import concourse.bass as bass
    import concourse.mybir as mybir
    import concourse.tile as tile

    f32 = mybir.dt.float32
    bf16 = mybir.dt.bfloat16

    nc = bacc.Bacc(
        "TRN2",
        target_bir_lowering=False,
        debug=False,
        num_devices=NCORES,
    )

    inA_d = nc.dram_tensor("inA", [128, WA], bf16, kind="ExternalInput")
    inB_d = nc.dram_tensor("inB", [128, WB], bf16, kind="ExternalInput")
    inC_d = nc.dram_tensor("inC", [128, WC], bf16, kind="ExternalInput")
    out_d = nc.dram_tensor("out", [128, NT * C], bf16, kind="ExternalOutput")

    inA = inA_d.ap()
    inB = inB_d.ap()
    inC = inC_d.ap()
    out = out_d.ap()

    with tile.TileContext(nc) as tc:
        with (
            tc.tile_pool(name="const", bufs=1) as cp,
            tc.tile_pool(name="io", bufs=4) as io,
            tc.tile_pool(name="psw", bufs=2, space="PSUM") as psw,
            tc.tile_pool(name="pso", bufs=4, space="PSUM") as pso,
        ):
            sbA = cp.tile([128, WA], bf16)
            sbB = cp.tile([128, WB], bf16)
            sbC = cp.tile([128, WC], bf16)

            # weights alone at each ring head; wpt first (its ring also
            # carries x1, needed earliest in PAIR_ORDER)
            nc.scalar.dma_start(out=sbB[:, :512], in_=inB[:, :512])      # wpt
            nc.sync.dma_start(out=sbA[:, :512], in_=inA[:, :512])        # wv
            nc.gpsimd.dma_start(out=sbC[:, :512], in_=inC[:, :512])      # x2
            nc.scalar.dma_start(out=sbB[:, 512:], in_=inB[:, 512:])      # x1
            nc.sync.dma_start(out=sbA[:, 512:], in_=inA[:, 512:])        # bias+x0
            nc.gpsimd.dma_start(out=sbC[:, 512:], in_=inC[:, 512:])      # x3

            # PE warmup: keeps the PE busy (HAM busy-window) until the fold
            warm_sb = cp.tile([128, 128], bf16)
            nc.vector.memset(warm_sb, 0.0)
            warm_ps = psw.tile([128, 128], f32, tag="warm")
            for _ in range(NWARM):
                nc.tensor.matmul(warm_ps, warm_sb, warm_sb, start=True, stop=True)

            # fold W2T[k,p] = sum_vd Wv[vd,k] * wpt[vd,p]
            # separate PSUM tile per k-chunk; casts on the ACT engine
            w2t = cp.tile([128, 2 * C], bf16)     # [p(k), kc*256 + pcol]
            for kc in range(2):
                ps_f = psw.tile([128, C], f32)
                for vdc in range(2):
                    nc.tensor.matmul(
                        ps_f,
                        sbA[:, vdc * C + kc * 128: vdc * C + kc * 128 + 128],
                        sbB[:, vdc * C:(vdc + 1) * C],
                        start=(vdc == 0),
                        stop=(vdc == 1),
                    )
                nc.scalar.copy(w2t[:, kc * C:(kc + 1) * C], ps_f)

            # main GEMM: one PSUM bank per pair; kc0 matmuls for both
            # halves first so only the kc0 cast gates the pipeline
            bias = sbA[:, A_BIAS:A_BIAS + C]
            bias_bc = bass.AP(
                tensor=bias.tensor,
                offset=bias.offset,
                ap=[list(bias.ap[0]), [0, 2], list(bias.ap[1])],
            )  # [128, 2(x0-stride), 256] broadcast view
            out_v = out.rearrange("p (t m) -> p t m", t=NT)
            for i, pr in enumerate(PAIR_ORDER):
                ps = pso.tile([128, 2, C], f32)
                for half in range(2):
                    for kc in range(2):
                        t = 2 * pr + half
                        blob, base = X_LOC[t // 2]
                        sb = {"A": sbA, "B": sbB, "C": sbC}[blob]
                        col = base + kc * CS + (t % 2) * 128
                        nc.tensor.matmul(
                            ps[:, half, :],
                            sb[:, col:col + 128],
                            w2t[:, kc * C:(kc + 1) * C],
                            start=(kc == 0),
                            stop=(kc == 1),
                        )
                ot = io.tile([128, 2, C], bf16)
                if FUSED_BIAS:
                    nc.vector.tensor_add(ot, ps, bias_bc)
                else:
                    for half in range(2):
                        nc.vector.tensor_add(ot[:, half, :], ps[:, half, :], bias)
                eng = nc.sync if i % 2 == 0 else nc.scalar
                eng.dma_start(out=out_v[:, 2 * pr:2 * pr + 2, :], in_=ot)

    nc.compile()
    return nc


def run_sharded(inputs, trace=False, trace_cores=None):
    """Shard inputs, run on the 8 NeuronCores, gather.  Returns
    (full_output, BassKernelResults)."""
    from concourse.bass_utils import run_bass_kernel_spmd

    x = np.asarray(inputs["x"], dtype=np.float32)
    w_qkv = np.asarray(inputs["w_qkv"], dtype=np.float32)
    w_proj = np.asarray(inputs["w_proj"], dtype=np.float32)
    b_proj = np.asarray(inputs["b_proj"], dtype=np.float32)

    if "nc" not in _cache:
        _cache["nc"] = _build()
    nc = _cache["nc"]

    # host-side layout marshaling + bf16 cast only (no FLOPs)
    xT = x.reshape(ROWS, C).T.astype(BF16)                   # [256, 8192]

    def img(w):  # [vd, c] (vdc-major) -> SBUF image [128, 512]
        return w.reshape(2, 128, C).transpose(1, 0, 2).reshape(128, 2 * C)

    wv_img = img(w_qkv[2 * C:3 * C].astype(BF16))
    wpt_img = img(np.ascontiguousarray(w_proj.T).astype(BF16))
    bias16 = np.broadcast_to(b_proj.astype(BF16), (128, C))

    in_maps = []
    for c in range(NCORES):
        xc = xT[:, c * RPC:(c + 1) * RPC]                    # [256, 1024]
        chunks = [
            xc[:, ch * CS:(ch + 1) * CS]
            .reshape(2, 128, CS).transpose(1, 0, 2).reshape(128, 2 * CS)
            for ch in range(4)
        ]
        inA = np.ascontiguousarray(
            np.concatenate([wv_img, bias16, chunks[0]], axis=1))
        inB = np.ascontiguousarray(np.concatenate([wpt_img, chunks[1]], axis=1))
        inC = np.ascontiguousarray(np.concatenate([chunks[2], chunks[3]], axis=1))
        in_maps.append({"inA": inA, "inB": inB, "inC": inC})

    res = run_bass_kernel_spmd(
        nc,
        in_maps,
        core_ids=list(range(NCORES)),
        trace=trace,
        trace_cores=trace_cores,
    )
    # out image [128, t, m] -> rows t*128+p of the core's [1024, 256] block
    blocks = [
        res.results[c]["out"].reshape(128, NT, C).transpose(1, 0, 2).reshape(RPC, C)
        for c in range(NCORES)
    ]
    out = np.concatenate(blocks, axis=0).astype(np.float32)  # [8192, 256]
    return out.reshape(B, N, C), res


def kernel(x, w_qkv, w_proj, b_proj, temperature):
    out, _ = run_sharded(
        {"x": x, "w_qkv": w_qkv, "w_proj": w_proj, "b_proj": b_proj}
    )
    return out
